# revision 1
# baseline (speedup 1.0000x reference)
"""Trainium2 Bass kernel for the DNL (disentangled non-local) attention block.

Reference computation (per batch b, with xf = x.reshape(B, C, N), N = H*W):
    q  = (wq @ xf + bq)  centered over n          [N, 32]
    k  = (wk @ xf + bk)  centered over n          [32, N]
    A  = softmax_rows(q @ k)                      [N, N]
    v  = relu(wv @ xf + bv)                       [C, N]
    mask = softmax(wm @ xf + bm)                  [N]
    tissue[c, m] = sum_n v[c, n] * (A[m, n] + mask[n])
    return (x, tissue)

Math simplifications used (all exact):
  - q/k biases, bm, and k-centering add per-row constants inside the row
    softmax and drop out; only q-centering survives (as "-mean_n q").
  - The mask term is a rank-1 correction vm[c] = sum_n v[c,n] mask[n].
  - No max-subtraction in softmax: |energy| <= ~5 for these input scales.

Device layout (per core; 8 cores = 4 batches x 2 query-halves of 2048):
  - E^T[j, m] = K[:, j]^T @ Qc^T[:, m] computed j-partitioned so that the
    softmax denominator (colsum) and the AV matmul both consume it without
    any transposes.  O[c, m] = V^T-blocks^T @ expE^T accumulates in PSUM.
  - The per-core query half is selected by permuting the spatial columns of
    the input on the host (j-sums are permutation invariant).
"""

import sys

import numpy as np

if "/opt/trn_rl_repo" not in sys.path:
    sys.path.insert(0, "/opt/trn_rl_repo")

import concourse.bacc as bacc
import concourse.bass as bass
import concourse.mybir as mybir
import concourse.tile as tile
from concourse.bass_utils import run_bass_kernel_spmd

F32 = mybir.dt.float32
F32R = mybir.dt.float32r
BF16 = mybir.dt.bfloat16
AF = mybir.ActivationFunctionType

# engine that accumulates the softmax denominator partials:
#   "pe"     - ones-matmul colsum passes over each exp tile (3rd PE pass)
#   "gpsimd" - elementwise adds on the otherwise-idle GpSimd engine
#   "dve"    - elementwise adds on the Vector engine
S_MODE = "gpsimd"

B, C, H, W = 4, 256, 64, 64
N = H * W          # 4096 spatial positions
D = 32             # C // 8, q/k channel dim
M = N // 2         # query rows per core (2048)
NB = N // 128      # 32 j-blocks
NMC = M // 512     # 4 m-chunks per core
N_CORES = 8
# c-split for the AV matmuls: 96 + 96 + (64 | s-row at partition 96).
# The third group leaves column-groups 2-3 of the PE array idle, so the
# softmax-denominator colsum (M=1) col-packs into the same pass for free.
C_SPLITS = [(0, 128), (128, 128)]

# fp32r = fp32 bit layout, reduced-precision PE path (1 cyc/row vs 4 for
# fp32 when the moving dim >= 256).  walrus requires every fp32r-matmul
# operand's memory location to be typed float32r, so all matmul-feeding
# tiles below are F32R; compute-engine writes round into them.


def build_nc():
    nc = bacc.Bacc("TRN2", target_bir_lowering=False)

    x_d = nc.dram_tensor("x", [C, N], F32R, kind="ExternalInput")
    wqt_d = nc.dram_tensor("wqt", [C, D], F32R, kind="ExternalInput")
    wkt_d = nc.dram_tensor("wkt", [C, D], F32R, kind="ExternalInput")
    wvt_d = nc.dram_tensor("wvt", [C, C], F32R, kind="ExternalInput")
    bv_d = nc.dram_tensor("bv", [1, C], F32R, kind="ExternalInput")
    wmt_d = nc.dram_tensor("wmt", [C, 1], F32R, kind="ExternalInput")
    out_d = nc.dram_tensor("out", [C, M], F32, kind="ExternalOutput")

    with tile.TileContext(nc) as tc, nc.allow_low_precision(
        reason="fp32r matmul operands are a deliberate precision/speed trade"
    ):
        with (
            tc.tile_pool(name="const", bufs=1) as cpool,
            tc.tile_pool(name="work", bufs=1) as wpool,
            tc.tile_pool(name="norm", bufs=2) as npool,
            tc.tile_pool(name="expsb", bufs=3) as epool,
            tc.tile_pool(name="osb", bufs=2) as opool,
        ):
            # ---------------- stage A: load + projections ----------------
            xf0 = cpool.tile([128, N], F32R, tag="xf0")
            xf1 = cpool.tile([128, N], F32R, tag="xf1")
            for t in range(4):
                nc.sync.dma_start(
                    xf0[:, bass.ts(t, 1024)], x_d[0:128, bass.ts(t, 1024)]
                )
                nc.sync.dma_start(
                    xf1[:, bass.ts(t, 1024)], x_d[128:256, bass.ts(t, 1024)]
                )

            # weights: [C, *] -> [128, 2, *] (channel blocks side by side)
            wqt = cpool.tile([128, 2, D], F32R, tag="wqt")
            wkt = cpool.tile([128, 2, D], F32R, tag="wkt")
            wvt = cpool.tile([128, 2, C], F32R, tag="wvt")
            wmt = cpool.tile([128, 2, 1], F32R, tag="wmt")
            bv = cpool.tile([1, C], F32R, tag="bv")
            nc.sync.dma_start(wqt[:], wqt_d.rearrange("(b p) d -> p b d", p=128))
            nc.sync.dma_start(wkt[:], wkt_d.rearrange("(b p) d -> p b d", p=128))
            nc.sync.dma_start(wvt[:], wvt_d.rearrange("(b p) d -> p b d", p=128))
            nc.sync.dma_start(wmt[:], wmt_d.rearrange("(b p) d -> p b d", p=128))
            nc.sync.dma_start(bv[:], bv_d[:])

            ones_colf = cpool.tile([128, 1], F32, tag="ones_colf")
            ones_rowf = cpool.tile([1, 128], F32, tag="ones_rowf")
            nc.vector.memset(ones_colf[:], 1.0)
            nc.vector.memset(ones_rowf[:], 1.0)
            ones_col = cpool.tile([128, 1], BF16, tag="ones_col")
            ones_row = cpool.tile([1, 128], F32R, tag="ones_row")
            nc.vector.tensor_copy(ones_col[:], ones_colf[:])
            nc.vector.tensor_copy(ones_row[:], ones_rowf[:])

            xfs = [xf0, xf1]

            k_sb = cpool.tile([2 * D, N], F32R, tag="k_sb")
            qct = cpool.tile([2 * D, N], F32R, tag="qct")
            vt_sb = cpool.tile([128, NB * C], BF16, tag="vt_sb")
            mask_col = cpool.tile([128, NB], BF16, tag="mask_col")
            vm_col = cpool.tile([128, 2], F32, tag="vm_col")

            with (
                tc.tile_pool(name="psA", bufs=2, space="PSUM") as psA,
                tc.tile_pool(name="psB", bufs=2, space="PSUM") as psB,
                tc.tile_pool(name="psSa", bufs=1, space="PSUM") as psSa,
            ):
                # K = wk @ xf  -> [32, N] (no bias needed)
                qt_sb = wpool.tile([D, N], F32, tag="qt_sb")
                for t in range(8):
                    kp = psA.tile([D, 512], F32, tag="kq_ps")
                    for cb in range(2):
                        nc.tensor.matmul(
                            kp[:],
                            wkt[:, cb, :],
                            xfs[cb][:, bass.ts(t, 512)],
                            start=(cb == 0),
                            stop=(cb == 1),
                        )
                    nc.scalar.copy(k_sb[0:D, bass.ts(t, 512)], kp[:])
                    qp = psA.tile([D, 512], F32, tag="kq_ps")
                    for cb in range(2):
                        nc.tensor.matmul(
                            qp[:],
                            wqt[:, cb, :],
                            xfs[cb][:, bass.ts(t, 512)],
                            start=(cb == 0),
                            stop=(cb == 1),
                        )
                    nc.scalar.copy(qt_sb[:, bass.ts(t, 512)], qp[:])

                # center q over n:  qc = q - mean_n(q)
                qsum = wpool.tile([D, 1], F32, tag="qsum")
                nc.vector.reduce_sum(qsum[:], qt_sb[:], axis=mybir.AxisListType.X)
                qneg = wpool.tile([D, 1], F32, tag="qneg")
                nc.scalar.mul(qneg[:], qsum[:], -1.0 / N)
                nc.vector.tensor_scalar_add(qct[0:D, :], qt_sb[:], qneg[:])
                # replicate K and Qc to partitions 32-63 for row-packed E_T
                nc.sync.dma_start(k_sb[D : 2 * D, :], k_sb[0:D, :])
                nc.sync.dma_start(qct[D : 2 * D, 0:M], qct[0:D, 0:M])

                # V^T[n, c] = relu(xf^T @ wv^T + bv), stored j-block-major
                for jb in range(NB):
                    vp = psB.tile([128, C], F32, tag="v_ps")
                    for cb in range(2):
                        nc.tensor.matmul(
                            vp[:],
                            xfs[cb][:, bass.ts(jb, 128)],
                            wvt[:, cb, :],
                            start=(cb == 0),
                            stop=False,
                        )
                    nc.tensor.matmul(
                        vp[:], ones_row[:], bv[:],
                        start=False, stop=True,
                    )
                    nc.vector.tensor_scalar_max(
                        vt_sb[:, bass.ts(jb, C)], vp[:], 0.0
                    )

                # pm = wm @ xf -> [1, N]; mask = softmax(pm) (bm drops out)
                pm_sb = wpool.tile([1, N], F32, tag="pm_sb")
                for t in range(8):
                    pp = psSa.tile([1, 512], F32, tag="pm_ps")
                    for cb in range(2):
                        nc.tensor.matmul(
                            pp[:],
                            wmt[:, cb, :],
                            xfs[cb][:, bass.ts(t, 512)],
                            start=(cb == 0),
                            stop=(cb == 1),
                        )
                    nc.scalar.copy(pm_sb[:, bass.ts(t, 512)], pp[:])
                epm_sb = wpool.tile([1, N], F32, tag="epm_sb")
                nc.scalar.activation(epm_sb[:], pm_sb[:], AF.Exp)
                zt = wpool.tile([1, 1], F32, tag="zt")
                nc.vector.reduce_sum(zt[:], epm_sb[:], axis=mybir.AxisListType.X)
                rz = wpool.tile([1, 1], F32, tag="rz")
                nc.vector.reciprocal(rz[:], zt[:])

                # unnormalized mask as a column tile [128, NB]: 32 tiny PE
                # transposes of [1, 128] slices (internal DRAM tiles fail to
                # load on this runtime, so no DRAM round trip)
                mcp = psSa.tile([128, NB], F32, tag="mc_ps")
                for jb in range(NB):
                    nc.tensor.transpose(
                        mcp[:, jb : jb + 1],
                        epm_sb[0:1, bass.ts(jb, 128)],
                        ones_rowf[0:1, 0:1],
                    )
                nc.vector.tensor_copy(mask_col[:], mcp[:])

                # vm_row[1, c] = sum_n V^T[n, c] * mask_u[n] (normalized by rz)
                vmp = psSa.tile([1, C], F32, tag="vm_ps")
                for jb in range(NB):
                    nc.tensor.matmul(
                        vmp[:],
                        mask_col[:, jb : jb + 1],
                        vt_sb[:, bass.ts(jb, C)],
                        start=(jb == 0),
                        stop=(jb == NB - 1),
                    )
                vm_row = wpool.tile([1, C], F32, tag="vm_row")
                nc.vector.tensor_scalar_mul(vm_row[:], vmp[:], rz[:])

                # vm as per-partition columns for the 96/96/64 c-split
                vcp = psSa.tile([128, 2], F32, tag="vc_ps")
                for ci, (c0, cn) in enumerate(C_SPLITS):
                    nc.tensor.transpose(
                        vcp[0:cn, ci : ci + 1],
                        vm_row[0:1, c0 : c0 + cn],
                        ones_rowf[0:1, 0:1],
                    )
                for ci, (c0, cn) in enumerate(C_SPLITS):
                    nc.vector.tensor_copy(
                        vm_col[0:cn, ci : ci + 1], vcp[0:cn, ci : ci + 1]
                    )

            # ---------------- main loop: attention ----------------
            # Software-pipelined emission: the packed E^T matmul pair for
            # group i+1 is emitted right after exp(i), so the PE never waits
            # on the Activation engine's exp in steady state.
            NG = NB // 2

            with (
                tc.tile_pool(name="psE", bufs=2, space="PSUM") as psE,
                tc.tile_pool(name="psO", bufs=1, space="PSUM") as psO,
                tc.tile_pool(name="psS", bufs=1, space="PSUM") as psS,
                tc.tile_pool(name="psR", bufs=1, space="PSUM") as psR,
            ):

                def emit_e(mc, g):
                    # two K=32 matmuls packed into row groups (0,0)/(32,0);
                    # they run concurrently in the PE array
                    e_ps = psE.tile([128, 1024], F32, tag="e_ps", name="e_ps")
                    for h in range(2):
                        jb = 2 * g + h
                        nc.tensor.matmul(
                            e_ps[:, bass.ts(h, 512)],
                            k_sb[h * D : (h + 1) * D, bass.ts(jb, 128)],
                            qct[h * D : (h + 1) * D, bass.ts(mc, 512)],
                            start=True,
                            stop=True,
                            tile_position=(h * D, 0),
                        )
                    return e_ps

                e_pending = emit_e(0, 0)
                s_eng = {"gpsimd": nc.gpsimd, "dve": nc.vector}.get(S_MODE)
                for mc in range(NMC):
                    o_ps = [
                        psO.tile([128, 512], F32, tag=f"o_ps{ci}", name=f"o_ps{ci}")
                        for ci in range(2)
                    ]
                    s_ps = psS.tile([1, 512], F32, tag="s_ps", name="s_ps")
                    if s_eng is not None:
                        s_acc = wpool.tile(
                            [128, 512], F32, tag="s_acc", bufs=2, name="s_acc"
                        )
                        s_eng.memset(s_acc[:], 0.0)
                    for g in range(NG):
                        e_sb = epool.tile([128, 1024], BF16, tag="e_sb", name="e_sb")
                        nc.scalar.activation(e_sb[:], e_pending[:], AF.Exp)
                        if g + 1 < NG:
                            e_pending = emit_e(mc, g + 1)
                        elif mc + 1 < NMC:
                            e_pending = emit_e(mc + 1, 0)
                        for h in range(2):
                            jb = 2 * g + h
                            first = jb == 0
                            last = jb == NB - 1
                            e_half = e_sb[:, bass.ts(h, 512)]
                            for ci, (c0, cn) in enumerate(C_SPLITS):
                                nc.tensor.matmul(
                                    o_ps[ci][0:cn, :],
                                    vt_sb[:, jb * C + c0 : jb * C + c0 + cn],
                                    e_half,
                                    start=first,
                                    stop=last,
                                )
                            if s_eng is None:
                                nc.tensor.matmul(
                                    s_ps[:],
                                    ones_col[:, 0:1],
                                    e_half,
                                    start=first,
                                    stop=last,
                                )
                            else:
                                s_eng.tensor_add(s_acc[:], s_acc[:], e_half)

                    # normalize: out[c, m] = O[c, m] / s[m] + vm[c].
                    # Copy O out of PSUM immediately so the banks free up for
                    # the next m-chunk's accumulation.
                    o_raw = []
                    for ci in range(2):
                        orw = opool.tile([128, 512], F32, tag=f"o_raw{ci}",
                                         name=f"o_raw{ci}")
                        nc.vector.tensor_copy(orw[:], o_ps[ci][:])
                        o_raw.append(orw)
                    if s_eng is not None:
                        # partition-reduce the accumulated rows on the PE
                        nc.tensor.matmul(
                            s_ps[:], ones_colf[:], s_acc[:],
                            start=True, stop=True,
                        )
                    rs_row = npool.tile([1, 512], F32R, tag="rs_row")
                    nc.vector.reciprocal(rs_row[:], s_ps[:])
                    rb_ps = psR.tile([128, 512], F32, tag="rb_ps", name="rb_ps")
                    nc.tensor.matmul(
                        rb_ps[:], ones_row[:], rs_row[:],
                        start=True, stop=True,
                    )
                    rb_sb = npool.tile([128, 512], F32, tag="rb_sb")
                    nc.vector.tensor_copy(rb_sb[:], rb_ps[:])
                    for ci, (c0, cn) in enumerate(C_SPLITS):
                        t_sb = opool.tile([128, 512], F32, tag="t_sb", name="t_sb")
                        nc.vector.tensor_mul(
                            t_sb[0:cn, :], o_raw[ci][0:cn, :], rb_sb[0:cn, :]
                        )
                        o_sb = opool.tile([128, 512], F32, tag="o_sb", name="o_sb")
                        nc.vector.tensor_scalar_add(
                            o_sb[0:cn, :], t_sb[0:cn, :], vm_col[0:cn, ci : ci + 1]
                        )
                        nc.sync.dma_start(
                            out_d[c0 : c0 + cn, bass.ts(mc, 512)],
                            o_sb[0:cn, :],
                        )

    nc.compile()
    return nc


_NC_CACHE = {}


def _get_nc():
    if "nc" not in _NC_CACHE:
        _NC_CACHE["nc"] = build_nc()
    return _NC_CACHE["nc"]


def build_in_maps(x, wq, bq, wk, bk, wv, bv, wm, bm):
    x = np.ascontiguousarray(np.asarray(x, dtype=np.float32))
    xf = x.reshape(B, C, N)
    wqt = np.ascontiguousarray(np.asarray(wq, np.float32).T)
    wkt = np.ascontiguousarray(np.asarray(wk, np.float32).T)
    wvt = np.ascontiguousarray(np.asarray(wv, np.float32).T)
    wmt = np.ascontiguousarray(np.asarray(wm, np.float32).T)
    bv2 = np.ascontiguousarray(np.asarray(bv, np.float32).reshape(1, C))

    in_maps = []
    for core in range(N_CORES):
        b, half = divmod(core, 2)
        if half == 0:
            xin = xf[b]
        else:
            # own query half first; j-sums are permutation invariant
            xin = np.concatenate([xf[b][:, M:], xf[b][:, :M]], axis=1)
        in_maps.append(
            {
                "x": np.ascontiguousarray(xin),
                "wqt": wqt,
                "wkt": wkt,
                "wvt": wvt,
                "bv": bv2,
                "wmt": wmt,
            }
        )
    return x, in_maps


def kernel(x, wq, bq, wk, bk, wv, bv, wm, bm):
    x, in_maps = build_in_maps(x, wq, bq, wk, bk, wv, bv, wm, bm)

    res = run_bass_kernel_spmd(_get_nc(), in_maps, list(range(N_CORES)))
    _NC_CACHE["last_results"] = res

    tissue = np.empty((B, C, N), np.float32)
    for core in range(N_CORES):
        b, half = divmod(core, 2)
        tissue[b][:, half * M : (half + 1) * M] = res.results[core]["out"]
    return x, tissue.reshape(B, C, H, W)



# revision 5
# speedup vs baseline: 1.2966x; 1.2966x over previous
"""Trainium2 Bass kernel for the DNL (disentangled non-local) attention block.

Reference computation (per batch b, with xf = x.reshape(B, C, N), N = H*W):
    q  = (wq @ xf + bq)  centered over n          [N, 32]
    k  = (wk @ xf + bk)  centered over n          [32, N]
    A  = softmax_rows(q @ k)                      [N, N]
    v  = relu(wv @ xf + bv)                       [C, N]
    mask = softmax(wm @ xf + bm)                  [N]
    tissue[c, m] = sum_n v[c, n] * (A[m, n] + mask[n])
    return (x, tissue)

Math simplifications used (all exact):
  - q/k biases, bm, and k-centering add per-row constants inside the row
    softmax and drop out; only q-centering survives (as "-mean_n q").
  - The mask term is a rank-1 correction vm[c] = sum_n v[c,n] mask[n].
  - No max-subtraction in softmax: |energy| <= ~5 for these input scales.

Device layout (per core; 8 cores = 4 batches x 2 query-halves of 2048):
  - E^T[j, m] = K[:, j]^T @ Qc^T[:, m] computed j-partitioned (bf16, 2-way
    row-packed) so the softmax denominator and AV matmul consume it without
    transposes.
  - exp(E^T) is written as fp8e4 and the AV matmul runs in fp8 DoubleRow
    mode (2 j-blocks = K=256 per pass at 0.5 cyc/row).  The V^T stationary
    carries a 257th all-ones column, so the softmax denominator colsum
    falls out of the same DoubleRow passes as a 1-partition output.
  - The pm (mask logit) projection rides along the V projection as a
    257th output column, giving pm already j-partitioned for the vm
    (rank-1 mask correction) matmuls; no PE transposes of pm needed.
  - The per-core query half is selected by permuting the spatial columns
    of the input on the host (j-sums are permutation invariant).
"""

import sys

import numpy as np

if "/opt/trn_rl_repo" not in sys.path:
    sys.path.insert(0, "/opt/trn_rl_repo")

import concourse.bacc as bacc
import concourse.bass as bass
import concourse.mybir as mybir
import concourse.tile as tile
from concourse.bass_utils import run_bass_kernel_spmd

F32 = mybir.dt.float32
F32R = mybir.dt.float32r
BF16 = mybir.dt.bfloat16
FP8 = mybir.dt.float8e4
AF = mybir.ActivationFunctionType
DR = mybir.MatmulPerfMode.DoubleRow

B, C, H, W = 4, 256, 64, 64
N = H * W          # 4096 spatial positions
D = 32             # C // 8, q/k channel dim
M = N // 2         # query rows per core (2048)
NB = N // 128      # 32 j-blocks
NG = NB // 2       # 16 j-block pairs (DoubleRow consumes 2 blocks/pass)
NMC = M // 512     # 4 m-chunks per core
CP = 272           # vt free width: 256 ch + ones col @256, padded to a
                   # 16B-aligned DoubleRow k-tile stride (fp8 Ldweights ISA)
N_CORES = 8


def build_nc():
    nc = bacc.Bacc("TRN2", target_bir_lowering=False)

    x_d = nc.dram_tensor("x", [128, 2, N], BF16, kind="ExternalInput")
    wqt_d = nc.dram_tensor("wqt", [128, 2, D], BF16, kind="ExternalInput")
    wkt_d = nc.dram_tensor("wkt", [128, 2, D], BF16, kind="ExternalInput")
    wvmt_d = nc.dram_tensor("wvmt", [128, 2, CP], BF16, kind="ExternalInput")
    bvm_d = nc.dram_tensor("bvm", [1, CP], BF16, kind="ExternalInput")
    out_d = nc.dram_tensor("out", [C, M], F32, kind="ExternalOutput")

    with tile.TileContext(nc) as tc, nc.allow_low_precision(
        reason="bf16/fp8 matmul operands are a deliberate precision/speed trade"
    ):
        with (
            tc.tile_pool(name="const", bufs=1) as cpool,
            tc.tile_pool(name="work", bufs=1) as wpool,
            tc.tile_pool(name="norm", bufs=2) as npool,
            tc.tile_pool(name="expsb", bufs=3) as epool,
            tc.tile_pool(name="osb", bufs=2) as opool,
        ):
            # ---------------- stage A: load + projections ----------------
            xsb = cpool.tile([128, 2, N], BF16, tag="xsb")
            for t in range(4):
                nc.sync.dma_start(
                    xsb[:, :, bass.ts(t, 1024)], x_d[:, :, bass.ts(t, 1024)]
                )

            wqt = cpool.tile([128, 2, D], BF16, tag="wqt")
            wkt = cpool.tile([128, 2, D], BF16, tag="wkt")
            wvmt = cpool.tile([128, 2, CP], BF16, tag="wvmt")
            bvm = cpool.tile([1, CP], BF16, tag="bvm")
            nc.sync.dma_start(wqt[:], wqt_d[:])
            nc.sync.dma_start(wkt[:], wkt_d[:])
            nc.sync.dma_start(wvmt[:], wvmt_d[:])
            nc.sync.dma_start(bvm[:], bvm_d[:])

            ones_colf = cpool.tile([128, 1], F32, tag="ones_colf")
            ones_rowf = cpool.tile([1, 128], F32, tag="ones_rowf")
            nc.vector.memset(ones_colf[:], 1.0)
            nc.vector.memset(ones_rowf[:], 1.0)
            ones_row_b = cpool.tile([1, 128], BF16, tag="ones_row_b")
            ones_row_r = cpool.tile([1, 128], F32R, tag="ones_row_r")
            ones_col_8 = cpool.tile([128, 1], FP8, tag="ones_col_8")
            nc.vector.tensor_copy(ones_row_b[:], ones_rowf[:])
            nc.vector.tensor_copy(ones_row_r[:], ones_rowf[:])
            nc.vector.tensor_copy(ones_col_8[:], ones_colf[:])

            # k/q replicated 2x along partitions for row-packed E^T matmuls
            k_sb = cpool.tile([2 * D, N], BF16, tag="k_sb")
            qct = cpool.tile([2 * D, M], BF16, tag="qct")
            vt_sb = cpool.tile([128, NB, CP], FP8, tag="vt_sb")
            pmt_col = cpool.tile([128, NB], F32, tag="pmt_col")
            epm_col = cpool.tile([128, NB, 16], FP8, tag="epm_col")
            vm_col = cpool.tile([128, 2], F32, tag="vm_col")

            # ones column of V^T (softmax denominator rides the AV matmul)
            nc.vector.memset(vt_sb[:, :, 256:257], 1.0)

            with (
                tc.tile_pool(name="psA", bufs=2, space="PSUM") as psA,
                tc.tile_pool(name="psB", bufs=2, space="PSUM") as psB,
                tc.tile_pool(name="psSa", bufs=1, space="PSUM") as psSa,
            ):
                # K = wk @ xf  -> [32, N] (no bias needed)
                qt_sb = wpool.tile([D, N], F32, tag="qt_sb")
                for t in range(8):
                    kp = psA.tile([D, 512], F32, tag="kq_ps")
                    for cb in range(2):
                        nc.tensor.matmul(
                            kp[:],
                            wkt[:, cb, :],
                            xsb[:, cb, bass.ts(t, 512)],
                            start=(cb == 0),
                            stop=(cb == 1),
                        )
                    nc.vector.tensor_copy(k_sb[0:D, bass.ts(t, 512)], kp[:])
                    qp = psA.tile([D, 512], F32, tag="kq_ps")
                    for cb in range(2):
                        nc.tensor.matmul(
                            qp[:],
                            wqt[:, cb, :],
                            xsb[:, cb, bass.ts(t, 512)],
                            start=(cb == 0),
                            stop=(cb == 1),
                        )
                    nc.vector.tensor_copy(qt_sb[:, bass.ts(t, 512)], qp[:])

                # center q over n:  qc = q - mean_n(q); only first M cols used
                qsum = wpool.tile([D, 1], F32, tag="qsum")
                nc.vector.reduce_sum(qsum[:], qt_sb[:], axis=mybir.AxisListType.X)
                qneg = wpool.tile([D, 1], F32, tag="qneg")
                nc.scalar.mul(qneg[:], qsum[:], -1.0 / N)
                nc.vector.tensor_scalar_add(
                    qct[0:D, :], qt_sb[:, 0:M], qneg[:]
                )
                # replicate K and Qc to partitions 32-63 for row-packed E_T
                nc.sync.dma_start(k_sb[D : 2 * D, :], k_sb[0:D, :])
                nc.sync.dma_start(qct[D : 2 * D, :], qct[0:D, :])

                # V^T[n, c] = relu(xf^T @ wv^T + bv), j-block-major, fp8.
                # Column 256 of the stationary is wm^T: pm^T rides along,
                # already j-partitioned (bm drops out of its softmax).
                for jb in range(NB):
                    vp = psB.tile([128, CP], F32, tag="v_ps")
                    for cb in range(2):
                        nc.tensor.matmul(
                            vp[:],
                            xsb[:, cb, bass.ts(jb, 128)],
                            wvmt[:, cb, :],
                            start=(cb == 0),
                            stop=False,
                        )
                    nc.tensor.matmul(
                        vp[:], ones_row_b[:], bvm[:],
                        start=False, stop=True,
                    )
                    nc.vector.tensor_scalar_max(
                        vt_sb[:, jb, 0:256], vp[:, 0:256], 0.0
                    )
                    nc.vector.tensor_copy(
                        pmt_col[:, jb : jb + 1], vp[:, 256:257]
                    )

                # mask = softmax(pm): exp (fp8), z = colsum via PE, vm =
                # sum_n V^T[n,c] * epm[n] / z  (rank-1 mask correction)
                nc.scalar.activation(epm_col[:, :, 0], pmt_col[:], AF.Exp)
                zp = psSa.tile([1, NB], F32, tag="z_ps")
                nc.tensor.matmul(
                    zp[:], ones_col_8[:], epm_col[:, :, 0], start=True, stop=True
                )
                zt = wpool.tile([1, 1], F32, tag="zt")
                nc.vector.reduce_sum(zt[:], zp[:], axis=mybir.AxisListType.X)
                rz = wpool.tile([1, 1], F32, tag="rz")
                nc.vector.reciprocal(rz[:], zt[:])

                vmp = psSa.tile([1, C], F32, tag="vm_ps")
                for g in range(NG):
                    nc.tensor.matmul(
                        vmp[:],
                        epm_col[:, 2 * g : 2 * g + 2, 0:1],
                        vt_sb[:, 2 * g : 2 * g + 2, 0:256],
                        start=(g == 0),
                        stop=(g == NG - 1),
                        perf_mode=DR,
                    )
                vm_row = wpool.tile([1, C], F32, tag="vm_row")
                nc.vector.tensor_scalar_mul(vm_row[:], vmp[:], rz[:])

                # vm as per-partition columns for the two 128-channel splits
                vcp = psSa.tile([128, 2], F32, tag="vc_ps")
                for ci in range(2):
                    nc.tensor.transpose(
                        vcp[:, ci : ci + 1],
                        vm_row[0:1, 128 * ci : 128 * (ci + 1)],
                        ones_rowf[0:1, 0:1],
                    )
                nc.vector.tensor_copy(vm_col[:], vcp[:])

            # ---------------- main loop: attention ----------------
            # Software-pipelined emission: the packed E^T matmul pair for
            # group i+1 is emitted right after exp(i), so the PE never waits
            # on the Activation engine's exp in steady state.
            with (
                tc.tile_pool(name="psE", bufs=2, space="PSUM") as psE,
                tc.tile_pool(name="psO", bufs=1, space="PSUM") as psO,
                tc.tile_pool(name="psS", bufs=1, space="PSUM") as psS,
                tc.tile_pool(name="psR", bufs=1, space="PSUM") as psR,
            ):

                def emit_e(mc, g):
                    # two K=32 bf16 matmuls packed into row groups (0,0)/(32,0)
                    e_ps = psE.tile([128, 2, 512], F32, tag="e_ps", name="e_ps")
                    for h in range(2):
                        jb = 2 * g + h
                        nc.tensor.matmul(
                            e_ps[:, h, :],
                            k_sb[h * D : (h + 1) * D, bass.ts(jb, 128)],
                            qct[h * D : (h + 1) * D, bass.ts(mc, 512)],
                            start=True,
                            stop=True,
                            tile_position=(h * D, 0),
                        )
                    return e_ps

                e_pending = emit_e(0, 0)
                for mc in range(NMC):
                    o_ps = [
                        psO.tile([128, 512], F32, tag=f"o_ps{ci}", name=f"o_ps{ci}")
                        for ci in range(2)
                    ]
                    s_ps = psS.tile([1, 512], F32, tag="s_ps", name="s_ps")
                    for g in range(NG):
                        e_sb = epool.tile([128, 2, 512], FP8, tag="e_sb",
                                          name="e_sb")
                        nc.scalar.activation(e_sb[:], e_pending[:], AF.Exp)
                        if g + 1 < NG:
                            e_pending = emit_e(mc, g + 1)
                        elif mc + 1 < NMC:
                            e_pending = emit_e(mc + 1, 0)
                        first = g == 0
                        last = g == NG - 1
                        for ci in range(2):
                            nc.tensor.matmul(
                                o_ps[ci][:],
                                vt_sb[:, 2 * g : 2 * g + 2, 128 * ci : 128 * (ci + 1)],
                                e_sb[:],
                                start=first,
                                stop=last,
                                perf_mode=DR,
                            )
                        nc.tensor.matmul(
                            s_ps[:],
                            vt_sb[:, 2 * g : 2 * g + 2, 256:257],
                            e_sb[:],
                            start=first,
                            stop=last,
                            perf_mode=DR,
                        )

                    # normalize: out[c, m] = O[c, m] / s[m] + vm[c]
                    rs_row = npool.tile([1, 512], F32R, tag="rs_row")
                    nc.vector.reciprocal(rs_row[:], s_ps[:])
                    rb_ps = psR.tile([128, 512], F32, tag="rb_ps", name="rb_ps")
                    nc.tensor.matmul(
                        rb_ps[:], ones_row_r[:], rs_row[:],
                        start=True, stop=True,
                    )
                    rb_sb = npool.tile([128, 512], F32, tag="rb_sb")
                    nc.vector.tensor_copy(rb_sb[:], rb_ps[:])
                    for ci in range(2):
                        t_sb = opool.tile([128, 512], F32, tag="t_sb", name="t_sb")
                        nc.vector.tensor_mul(t_sb[:], o_ps[ci][:], rb_sb[:])
                        o_sb = opool.tile([128, 512], F32, tag="o_sb", name="o_sb")
                        nc.gpsimd.tensor_scalar_add(
                            o_sb[:], t_sb[:], vm_col[:, ci : ci + 1]
                        )
                        nc.sync.dma_start(
                            out_d[128 * ci : 128 * (ci + 1), bass.ts(mc, 512)],
                            o_sb[:],
                        )

    nc.compile()
    return nc


_NC_CACHE = {}


def _get_nc():
    if "nc" not in _NC_CACHE:
        _NC_CACHE["nc"] = build_nc()
    return _NC_CACHE["nc"]


def build_in_maps(x, wq, bq, wk, bk, wv, bv, wm, bm):
    import ml_dtypes

    bf16 = ml_dtypes.bfloat16
    x = np.ascontiguousarray(np.asarray(x, dtype=np.float32))
    xf = x.reshape(B, C, N)

    def blocked(a):
        # [C, F] -> [128, 2, F] (channel block index in dim 1)
        f = a.shape[1]
        return np.ascontiguousarray(
            a.reshape(2, 128, f).transpose(1, 0, 2).astype(bf16)
        )

    wqt = blocked(np.asarray(wq, np.float32).T)
    wkt = blocked(np.asarray(wk, np.float32).T)
    wvm = np.concatenate(
        [
            np.asarray(wv, np.float32).T,
            np.asarray(wm, np.float32).T,
            np.zeros((C, CP - 257), np.float32),
        ],
        axis=1,
    )  # [C, CP]
    wvmt = blocked(wvm)
    bvm = np.concatenate(
        [np.asarray(bv, np.float32).reshape(C), np.zeros(CP - C, np.float32)]
    ).reshape(1, CP).astype(bf16)

    in_maps = []
    for core in range(N_CORES):
        b, half = divmod(core, 2)
        if half == 0:
            xin = xf[b]
        else:
            # own query half first; j-sums are permutation invariant
            xin = np.concatenate([xf[b][:, M:], xf[b][:, :M]], axis=1)
        xin = np.ascontiguousarray(
            xin.reshape(2, 128, N).transpose(1, 0, 2).astype(bf16)
        )
        in_maps.append(
            {
                "x": xin,
                "wqt": wqt,
                "wkt": wkt,
                "wvmt": wvmt,
                "bvm": bvm,
            }
        )
    return x, in_maps


def kernel(x, wq, bq, wk, bk, wv, bv, wm, bm):
    x, in_maps = build_in_maps(x, wq, bq, wk, bk, wv, bv, wm, bm)

    res = run_bass_kernel_spmd(_get_nc(), in_maps, list(range(N_CORES)))
    _NC_CACHE["last_results"] = res

    tissue = np.empty((B, C, N), np.float32)
    for core in range(N_CORES):
        b, half = divmod(core, 2)
        tissue[b][:, half * M : (half + 1) * M] = res.results[core]["out"]
    return x, tissue.reshape(B, C, H, W)


# revision 9
# speedup vs baseline: 1.5268x; 1.1776x over previous
"""Trainium2 Bass kernel for the DNL (disentangled non-local) attention block.

Reference computation (per batch b, with xf = x.reshape(B, C, N), N = H*W):
    q  = (wq @ xf + bq)  centered over n          [N, 32]
    k  = (wk @ xf + bk)  centered over n          [32, N]
    A  = softmax_rows(q @ k)                      [N, N]
    v  = relu(wv @ xf + bv)                       [C, N]
    mask = softmax(wm @ xf + bm)                  [N]
    tissue[c, m] = sum_n v[c, n] * (A[m, n] + mask[n])
    return (x, tissue)

Math simplifications used (all exact):
  - q/k biases, bm, and k-centering add per-row constants inside the row
    softmax and drop out; only q-centering survives (as "-mean_n q").
  - The mask term is a rank-1 correction vm[c] = sum_n v[c,n] mask[n].
  - No max-subtraction in softmax: |energy| <= ~5 for these input scales.

Device layout (per core; 8 cores = 4 batches x 2 query-halves of 2048):
  - All heavy matmuls run fp8e4 DoubleRow (2 k-tiles per pass, 0.5
    cyc/row).  Host-side weight scaling keeps fp8 operands in the normal
    range: wq,wk x4 (energy x16, undone by exp's scale=1/16), wv x16
    (undone by folding 1/16 into the 1/s broadcast row and into rz).
  - E^T[j, m] = K^T Qc^T j-partitioned, two j-blocks row-packed at PE row
    tiles 0/32 (K=16x2 fp8 DR each), so softmax denominator and AV matmul
    consume exp(E^T) without transposes.
  - The V^T fp8 stationary carries a 257th all-ones column: the softmax
    denominator colsum falls out of the AV DoubleRow passes for free.
  - The pm (mask logit) projection rides the V projection as column 256
    of the wvm stationary, giving pm already j-partitioned for the vm
    (rank-1 mask) matmuls; bm drops out of its softmax.
  - The per-core query half is selected by permuting the spatial columns
    of the input on the host (j-sums are permutation invariant).
"""

import sys

import numpy as np

if "/opt/trn_rl_repo" not in sys.path:
    sys.path.insert(0, "/opt/trn_rl_repo")

import concourse.bacc as bacc
import concourse.bass as bass
import concourse.mybir as mybir
import concourse.tile as tile
from concourse.bass_utils import run_bass_kernel_spmd

F32 = mybir.dt.float32
F32R = mybir.dt.float32r
BF16 = mybir.dt.bfloat16
FP8 = mybir.dt.float8e4
AF = mybir.ActivationFunctionType
DR = mybir.MatmulPerfMode.DoubleRow

B, C, H, W = 4, 256, 64, 64
N = H * W          # 4096 spatial positions
D = 32             # C // 8, q/k channel dim
M = N // 2         # query rows per core (2048)
NB = N // 128      # 32 j-blocks
NG = NB // 2       # 16 j-block pairs (DoubleRow consumes 2 blocks/pass)
NMC = M // 512     # 4 m-chunks per core
CP = 272           # vt free width: 256 ch + ones col @256, padded to a
                   # 16B-aligned DoubleRow k-tile stride (fp8 Ldweights ISA)
QS = 4.0           # host scale on wq and wk (energy x16)
VS = 16.0          # host scale on wv/bv/wm (v and pm x16)
RS = 1.0 / VS
N_CORES = 8


def build_nc():
    nc = bacc.Bacc("TRN2", target_bir_lowering=False)

    x_d = nc.dram_tensor("x", [128, 2, N], BF16, kind="ExternalInput")
    x8_d = nc.dram_tensor("x8", [128, 2, N], FP8, kind="ExternalInput")
    wqt_d = nc.dram_tensor("wqt", [128, 2, D], BF16, kind="ExternalInput")
    wkt_d = nc.dram_tensor("wkt", [128, 2, D], BF16, kind="ExternalInput")
    wvmt_d = nc.dram_tensor("wvmt", [128, 2, CP], FP8, kind="ExternalInput")
    bvm_d = nc.dram_tensor("bvm", [1, CP], BF16, kind="ExternalInput")
    out_d = nc.dram_tensor("out", [C, M], F32, kind="ExternalOutput")

    with tile.TileContext(nc) as tc, nc.allow_low_precision(
        reason="bf16/fp8 matmul operands are a deliberate precision/speed trade"
    ):
        with (
            tc.tile_pool(name="const", bufs=1) as cpool,
            tc.tile_pool(name="work", bufs=1) as wpool,
            tc.tile_pool(name="norm", bufs=2) as npool,
            tc.tile_pool(name="expsb", bufs=3) as epool,
            tc.tile_pool(name="osb", bufs=2) as opool,
        ):
            # ---------------- stage A: load + projections ----------------
            xsb = cpool.tile([128, 2, N], BF16, tag="xsb")
            x8 = cpool.tile([128, 2, N], FP8, tag="x8")
            for t in range(4):
                nc.sync.dma_start(
                    xsb[:, :, bass.ts(t, 1024)], x_d[:, :, bass.ts(t, 1024)]
                )
                nc.sync.dma_start(
                    x8[:, :, bass.ts(t, 1024)], x8_d[:, :, bass.ts(t, 1024)]
                )

            wqt = cpool.tile([128, 2, D], BF16, tag="wqt")
            wkt = cpool.tile([128, 2, D], BF16, tag="wkt")
            wvmt = cpool.tile([128, 2, CP], FP8, tag="wvmt")
            bvm = cpool.tile([1, CP], BF16, tag="bvm")
            nc.sync.dma_start(wqt[:], wqt_d[:])
            nc.sync.dma_start(wkt[:], wkt_d[:])
            nc.sync.dma_start(wvmt[:], wvmt_d[:])
            nc.sync.dma_start(bvm[:], bvm_d[:])

            ones_colf = cpool.tile([128, 1], F32, tag="ones_colf")
            ones_rowf = cpool.tile([1, 128], F32, tag="ones_rowf")
            nc.vector.memset(ones_colf[:], 1.0)
            nc.vector.memset(ones_rowf[:], 1.0)
            ones_row_b = cpool.tile([1, 128], BF16, tag="ones_row_b")
            ones_col_8 = cpool.tile([128, 1], FP8, tag="ones_col_8")
            nc.vector.tensor_copy(ones_row_b[:], ones_rowf[:])
            nc.vector.tensor_copy(ones_col_8[:], ones_colf[:])
            # 1/s broadcast stationary carries the 1/VS unscale of vt
            rsc_f = cpool.tile([1, 128], F32, tag="rsc_f")
            nc.vector.memset(rsc_f[:], RS)
            rsc_row = cpool.tile([1, 128], F32R, tag="rsc_row")
            nc.vector.tensor_copy(rsc_row[:], rsc_f[:])

            # k/q bf16 (fp8 energies cost too much accuracy), replicated 2x
            # along partitions for row-packed E^T matmuls
            k_sb = cpool.tile([2 * D, N], BF16, tag="k_sb")
            qct = cpool.tile([2 * D, M], BF16, tag="qct")
            vt_sb = cpool.tile([128, NB, CP], FP8, tag="vt_sb")
            pmt_col = cpool.tile([128, NB], F32, tag="pmt_col")
            epm_col = cpool.tile([128, NB, 16], FP8, tag="epm_col")
            vm_col = cpool.tile([128, 2], F32, tag="vm_col")

            # ones column of V^T (softmax denominator rides the AV matmul)
            nc.vector.memset(vt_sb[:, :, 256:257], 1.0)

            with (
                tc.tile_pool(name="psA", bufs=2, space="PSUM") as psA,
                tc.tile_pool(name="psB", bufs=2, space="PSUM") as psB,
                tc.tile_pool(name="psSa", bufs=1, space="PSUM") as psSa,
            ):
                # K = wk @ xf  -> [32, N] fp8 (no bias needed)
                qt_sb = wpool.tile([D, N], F32, tag="qt_sb")
                for t in range(8):
                    kp = psA.tile([D, 512], F32, tag="kq_ps")
                    for cb in range(2):
                        nc.tensor.matmul(
                            kp[:],
                            wkt[:, cb, :],
                            xsb[:, cb, bass.ts(t, 512)],
                            start=(cb == 0),
                            stop=(cb == 1),
                        )
                    nc.vector.tensor_copy(k_sb[0:D, bass.ts(t, 512)], kp[:])
                    qp = psA.tile([D, 512], F32, tag="kq_ps")
                    for cb in range(2):
                        nc.tensor.matmul(
                            qp[:],
                            wqt[:, cb, :],
                            xsb[:, cb, bass.ts(t, 512)],
                            start=(cb == 0),
                            stop=(cb == 1),
                        )
                    nc.vector.tensor_copy(qt_sb[:, bass.ts(t, 512)], qp[:])

                # center q over n:  qc = q - mean_n(q); only first M cols used
                qsum = wpool.tile([D, 1], F32, tag="qsum")
                nc.vector.reduce_sum(qsum[:], qt_sb[:], axis=mybir.AxisListType.X)
                qneg = wpool.tile([D, 1], F32, tag="qneg")
                nc.scalar.mul(qneg[:], qsum[:], -1.0 / N)
                nc.vector.tensor_scalar_add(
                    qct[0:D, :], qt_sb[:, 0:M], qneg[:]
                )
                # replicate K and Qc to partitions 32-63 for row-packed E_T
                nc.sync.dma_start(k_sb[D : 2 * D, :], k_sb[0:D, :])
                nc.sync.dma_start(qct[D : 2 * D, :], qct[0:D, :])

                # V^T[n, c] = relu(xf^T @ wv^T + bv) * VS, j-block-major, fp8.
                # Column 256 of the stationary is wm^T: pm^T rides along,
                # already j-partitioned.
                for jb in range(NB):
                    vp = psB.tile([128, CP], F32, tag="v_ps")
                    nc.tensor.matmul(
                        vp[:],
                        x8[:, :, bass.ts(jb, 128)],
                        wvmt[:],
                        start=True,
                        stop=False,
                        perf_mode=DR,
                    )
                    nc.tensor.matmul(
                        vp[:], ones_row_b[:], bvm[:],
                        start=False, stop=True,
                    )
                    nc.vector.tensor_scalar_max(
                        vt_sb[:, jb, 0:256], vp[:, 0:256], 0.0
                    )
                    nc.vector.tensor_copy(
                        pmt_col[:, jb : jb + 1], vp[:, 256:257]
                    )

                # mask = softmax(pm): exp undoes the VS scale; z = colsum via
                # PE; vm[c] = sum_n V^T[n,c] epm[n] / (VS * z)
                nc.scalar.activation(
                    epm_col[:, :, 0], pmt_col[:], AF.Exp, scale=RS
                )
                zp = psSa.tile([1, NB], F32, tag="z_ps")
                nc.tensor.matmul(
                    zp[:], ones_col_8[:], epm_col[:, :, 0], start=True, stop=True
                )
                zt = wpool.tile([1, 1], F32, tag="zt")
                nc.vector.reduce_sum(zt[:], zp[:], axis=mybir.AxisListType.X)
                rz = wpool.tile([1, 1], F32, tag="rz")
                nc.vector.reciprocal(rz[:], zt[:])

                vmp = psSa.tile([1, C], F32, tag="vm_ps")
                for g in range(NG):
                    nc.tensor.matmul(
                        vmp[:],
                        epm_col[:, 2 * g : 2 * g + 2, 0:1],
                        vt_sb[:, 2 * g : 2 * g + 2, 0:256],
                        start=(g == 0),
                        stop=(g == NG - 1),
                        perf_mode=DR,
                    )
                vm_row = wpool.tile([1, C], F32, tag="vm_row")
                nc.vector.tensor_scalar(
                    vm_row[:], vmp[:], rz[0:1, 0:1], RS,
                    mybir.AluOpType.mult, mybir.AluOpType.mult,
                )

                # vm as per-partition columns for the two 128-channel splits
                vcp = psSa.tile([128, 2], F32, tag="vc_ps")
                for ci in range(2):
                    nc.tensor.transpose(
                        vcp[:, ci : ci + 1],
                        vm_row[0:1, 128 * ci : 128 * (ci + 1)],
                        ones_rowf[0:1, 0:1],
                    )
                nc.vector.tensor_copy(vm_col[:], vcp[:])

            # ---------------- main loop: attention ----------------
            # Software-pipelined emission: the packed E^T matmul pair for
            # group i+1 is emitted right after exp(i), so the PE never waits
            # on the Activation engine's exp in steady state.
            with (
                tc.tile_pool(name="psE", bufs=2, space="PSUM") as psE,
                tc.tile_pool(name="psO", bufs=1, space="PSUM") as psO,
                tc.tile_pool(name="psS", bufs=1, space="PSUM") as psS,
                tc.tile_pool(name="psR", bufs=1, space="PSUM") as psR,
            ):

                def emit_e(mc, g):
                    # two K=32 bf16 matmuls packed at PE row tiles 0/32
                    e_ps = psE.tile([128, 2, 512], F32, tag="e_ps", name="e_ps")
                    for h in range(2):
                        jb = 2 * g + h
                        nc.tensor.matmul(
                            e_ps[:, h, :],
                            k_sb[h * D : (h + 1) * D, bass.ts(jb, 128)],
                            qct[h * D : (h + 1) * D, bass.ts(mc, 512)],
                            start=True,
                            stop=True,
                            tile_position=(h * D, 0),
                        )
                    return e_ps

                e_pending = emit_e(0, 0)
                for mc in range(NMC):
                    o_ps = [
                        psO.tile([128, 512], F32, tag=f"o_ps{ci}", name=f"o_ps{ci}")
                        for ci in range(2)
                    ]
                    s_ps = psS.tile([1, 512], F32, tag="s_ps", name="s_ps")
                    for g in range(NG):
                        e_sb = epool.tile([128, 2, 512], FP8, tag="e_sb",
                                          name="e_sb")
                        # scale=1/16 undoes the host x4 on wq and wk
                        nc.scalar.activation(
                            e_sb[:], e_pending[:], AF.Exp, scale=1.0 / 16.0
                        )
                        if g + 1 < NG:
                            e_pending = emit_e(mc, g + 1)
                        elif mc + 1 < NMC:
                            e_pending = emit_e(mc + 1, 0)
                        first = g == 0
                        last = g == NG - 1
                        for ci in range(2):
                            nc.tensor.matmul(
                                o_ps[ci][:],
                                vt_sb[:, 2 * g : 2 * g + 2, 128 * ci : 128 * (ci + 1)],
                                e_sb[:],
                                start=first,
                                stop=last,
                                perf_mode=DR,
                            )
                        nc.tensor.matmul(
                            s_ps[:],
                            vt_sb[:, 2 * g : 2 * g + 2, 256:257],
                            e_sb[:],
                            start=first,
                            stop=last,
                            perf_mode=DR,
                        )

                    # normalize: out[c, m] = O[c, m] / (VS * s[m]) + vm[c]
                    rs_f = npool.tile([1, 512], F32, tag="rs_f")
                    rs_scr = npool.tile([1, 512], F32, tag="rs_scr")
                    nc.vector.reciprocal_approx_accurate(
                        rs_f[:], s_ps[:], rs_scr[:]
                    )
                    rs_row = npool.tile([1, 512], F32R, tag="rs_row")
                    nc.vector.tensor_copy(rs_row[:], rs_f[:])
                    rb_ps = psR.tile([128, 512], F32, tag="rb_ps", name="rb_ps")
                    nc.tensor.matmul(
                        rb_ps[:], rsc_row[:], rs_row[:],
                        start=True, stop=True,
                    )
                    rb_sb = npool.tile([128, 512], F32, tag="rb_sb")
                    nc.vector.tensor_copy(rb_sb[:], rb_ps[:])
                    for ci in range(2):
                        t_sb = opool.tile([128, 512], F32, tag="t_sb", name="t_sb")
                        nc.vector.tensor_mul(t_sb[:], o_ps[ci][:], rb_sb[:])
                        o_sb = opool.tile([128, 512], F32, tag="o_sb", name="o_sb")
                        nc.vector.tensor_scalar_add(
                            o_sb[:], t_sb[:], vm_col[:, ci : ci + 1]
                        )
                        nc.sync.dma_start(
                            out_d[128 * ci : 128 * (ci + 1), bass.ts(mc, 512)],
                            o_sb[:],
                        )

    nc.compile()
    return nc


_NC_CACHE = {}


def _get_nc():
    if "nc" not in _NC_CACHE:
        _NC_CACHE["nc"] = build_nc()
    return _NC_CACHE["nc"]


def build_in_maps(x, wq, bq, wk, bk, wv, bv, wm, bm):
    import ml_dtypes

    bf16 = ml_dtypes.bfloat16
    fp8 = ml_dtypes.float8_e4m3
    x = np.ascontiguousarray(np.asarray(x, dtype=np.float32))
    xf = x.reshape(B, C, N)

    def blocked(a, dt):
        # [C, F] -> [128, 2, F] (channel block index in dim 1)
        f = a.shape[1]
        return np.ascontiguousarray(
            a.reshape(2, 128, f).transpose(1, 0, 2).astype(dt)
        )

    wqt = blocked(np.asarray(wq, np.float32).T * QS, bf16)
    wkt = blocked(np.asarray(wk, np.float32).T * QS, bf16)
    wvm = np.concatenate(
        [
            np.asarray(wv, np.float32).T * VS,
            np.asarray(wm, np.float32).T * VS,
            np.zeros((C, CP - 257), np.float32),
        ],
        axis=1,
    )  # [C, CP]
    wvmt = blocked(wvm, fp8)
    bvm = np.concatenate(
        [np.asarray(bv, np.float32).reshape(C) * VS, np.zeros(CP - C, np.float32)]
    ).reshape(1, CP).astype(bf16)

    in_maps = []
    for core in range(8):
        b, half = divmod(core, 2)
        if half == 0:
            xin = xf[b]
        else:
            # own query half first; j-sums are permutation invariant
            xin = np.concatenate([xf[b][:, M:], xf[b][:, :M]], axis=1)
        xin = np.ascontiguousarray(xin.reshape(2, 128, N).transpose(1, 0, 2))
        in_maps.append(
            {
                "x": np.ascontiguousarray(xin.astype(bf16)),
                "x8": np.ascontiguousarray(xin.astype(fp8)),
                "wqt": wqt,
                "wkt": wkt,
                "wvmt": wvmt,
                "bvm": bvm,
            }
        )
    return x, in_maps


def kernel(x, wq, bq, wk, bk, wv, bv, wm, bm):
    x, in_maps = build_in_maps(x, wq, bq, wk, bk, wv, bv, wm, bm)

    res = run_bass_kernel_spmd(_get_nc(), in_maps, list(range(8)))
    _NC_CACHE["last_results"] = res

    tissue = np.empty((B, C, N), np.float32)
    for core in range(8):
        b, half = divmod(core, 2)
        tissue[b][:, half * M : (half + 1) * M] = res.results[core]["out"]
    return x, tissue.reshape(B, C, H, W)
